# revision 5
# baseline (speedup 1.0000x reference)
"""2-layer 4-head GAT (DGL GATConv-style) as a distributed Bass/Tile kernel
on 8 Trainium2 NeuronCores.

Sharding: destination nodes split 6272/core (49 tiles of 128). Per layer every
core redundantly computes the dense projections for ALL nodes into a bf16
"dcat" table in its HBM (4 node-tiles per matmul group): row n =
[X@W (256) | X@w_el (4) | pad] at a 768B stride (w_el is host-prefolded W@al).
The edge phase then, per GROUP of two 128-destination tiles, dma_gathers the
[fs|el] rows by src (the ONLY per-edge gathers; two calls per group, one per
int16-indexable table half), and:
  - er[dst] per edge slot comes from a tiny per-chunk matmul with a
    host-streamed transposed one-hot indicator (indT, fp8) against the tile's
    er column (from the own-rows phase) -- no dst-side gather.
  - p = exp(leakyrelu(el_src + er_dst)); messages scaled by p on DVE;
  - scatter-add into PSUM [128, 260] via host-streamed one-hot indicator
    matmuls (ind, fp8) -- no on-chip indicator builds.
The per-tile epilogue (fused) normalizes by the attention sums, means heads,
adds the residual (own-rows phase) and LayerNorm+ReLU, then stores the tile
TRANSPOSED (h1ownT, bf16); layer-0 results are AllGathered as h1T blocks so
layer-1's dense phase needs no transposes; the layer-1 epilogue applies the
prediction head.

The "own-rows" phase computes er/res for the core's own 6272 nodes from
per-core pre-transposed inputs (ownxT for layer 0, h1ownT for layer 1), so no
per-core addressing of shared tables is needed anywhere.
"""
import contextlib
import ctypes
import os
import sys
import types

import numpy as np

sys.path.insert(0, "/opt/trn_rl_repo")

import ml_dtypes  # noqa: E402

# ---------------------------------------------------------------------------
# Shim: antenv.axon_hooks (missing in this image) so trace=True works.
# ---------------------------------------------------------------------------
_ntff_hook = None


def _install_axon_hooks_shim():
    global _ntff_hook
    if "antenv.axon_hooks" in sys.modules:
        return
    try:
        import antenv
    except ImportError:
        return
    mod = types.ModuleType("antenv.axon_hooks")

    def set_axon_ntff_profile_hook(h):
        global _ntff_hook
        _ntff_hook = h

    def get_axon_ntff_profile_hook():
        return _ntff_hook

    mod.set_axon_ntff_profile_hook = set_axon_ntff_profile_hook
    mod.get_axon_ntff_profile_hook = get_axon_ntff_profile_hook
    sys.modules["antenv.axon_hooks"] = mod
    antenv.axon_hooks = mod

    so_path = "/opt/axon/libaxon_pjrt.so"
    try:
        lib = ctypes.CDLL(so_path)
    except OSError:
        return
    if not hasattr(lib, "axon_start_nrt_profile"):
        return
    lib.axon_start_nrt_profile.argtypes = [
        ctypes.POINTER(ctypes.c_int64),
        ctypes.c_size_t,
    ]
    lib.axon_start_nrt_profile.restype = ctypes.c_int64
    lib.axon_stop_nrt_profile.argtypes = [ctypes.c_char_p]
    lib.axon_stop_nrt_profile.restype = ctypes.c_int64

    @contextlib.contextmanager
    def _hook(output_dir, device_ids):
        import jax

        jax.devices()
        if device_ids:
            ids = (ctypes.c_int64 * len(device_ids))(*device_ids)
            rc = lib.axon_start_nrt_profile(ids, len(device_ids))
        else:
            rc = lib.axon_start_nrt_profile(None, 0)
        if rc != 0:
            raise RuntimeError(f"axon_start_nrt_profile rc={rc}")
        try:
            yield
        finally:
            n = lib.axon_stop_nrt_profile(str(output_dir).encode())
            if n < 0:
                raise RuntimeError(f"axon_stop_nrt_profile rc={n}")
            print(f"profile: {n} file(s) written to {output_dir}", file=sys.stderr)

    set_axon_ntff_profile_hook(_hook)


_install_axon_hooks_shim()

import concourse.bass as bass  # noqa: E402
import concourse.bacc as bacc  # noqa: E402
import concourse.mybir as mybir  # noqa: E402
import concourse.tile as tile  # noqa: E402
from concourse.bass_utils import run_bass_kernel_spmd  # noqa: E402


# ---------------------------------------------------------------------------
# Problem constants (kernel.py is self-contained; shapes are hardcoded).
# ---------------------------------------------------------------------------
N, E = 50000, 800000
IN, HID, H, OUT = 128, 64, 4, 64
NEG_SLOPE = 0.2
EPS = 1e-5

P = 128
NCORES = 8
T = 49                       # dst node tiles per core
NPC = T * P                  # 6272 nodes per core
N_PAD = NCORES * NPC         # 50176 (node space)
NHALF = N_PAD // 2           # 25088 rows per dcat half (int16-indexable)
RL = 384                     # dcat row length (bf16) -> 768B stride
DCOLS = 260                  # occupied dcat cols: [fs 256 | el 4]
C_EL = 256
TINY = 1e-30
TILES_G = N_PAD // P         # 392 global tiles
GD = 4                       # dense-phase tiles per matmul group

# edge-phase tile groups (gathers/streams merged per group)
GROUPS = [[t, t + 1] for t in range(0, T - 1, 2)] + [[T - 1]]

F32 = mybir.dt.float32
BF16 = mybir.dt.bfloat16
FP8 = mybir.dt.float8e4
I16 = mybir.dt.int16
OP = mybir.AluOpType
AF = mybir.ActivationFunctionType
AX = mybir.AxisListType


def _cdiv(a, b):
    return (a + b - 1) // b


# ---------------------------------------------------------------------------
# Host-side edge preprocessing
# ---------------------------------------------------------------------------
def _wrap_idx(flat):
    """dma_gather index layout: idx j -> [j%16, j//16], replicated to 128
    partitions."""
    n = len(flat)
    assert n % 128 == 0
    cols = n // 16
    w = np.zeros((16, cols), np.int16)
    w[np.arange(n) % 16, np.arange(n) // 16] = flat
    return np.tile(w, (8, 1))


def _prep_edges(src, dst):
    src = np.asarray(src).astype(np.int64)
    dst = np.asarray(dst).astype(np.int64)
    order = np.argsort(dst, kind="stable")
    src = src[order]
    dst = dst[order]

    bounds = np.searchsorted(dst, np.arange(0, N_PAD + 1, P))

    lo_lists = [[None] * T for _ in range(NCORES)]
    hi_lists = [[None] * T for _ in range(NCORES)]
    for c in range(NCORES):
        for t in range(T):
            gt = c * T + t
            e0, e1 = bounds[gt], bounds[gt + 1]
            s = np.asarray(src[e0:e1])
            d = np.asarray(dst[e0:e1]) - gt * P
            is_lo = s < NHALF
            lo_lists[c][t] = (s[is_lo], d[is_lo])
            hi_lists[c][t] = (s[~is_lo] - NHALF, d[~is_lo])

    K_lo = [
        max(1, max(_cdiv(len(lo_lists[c][t][0]), P) for c in range(NCORES)))
        for t in range(T)
    ]
    K_hi = [
        max(1, max(_cdiv(len(hi_lists[c][t][0]), P) for c in range(NCORES)))
        for t in range(T)
    ]

    # group metadata: per group the idx col offset, chunk offset, and per-tile
    # chunk bases (lo chunks of all tiles first, then hi chunks).
    ginfo = []
    gio = 0
    gdl = 0
    for tl in GROUPS:
        skl = sum(K_lo[t] for t in tl)
        skh = sum(K_hi[t] for t in tl)
        lo_base = {}
        hi_base = {}
        b = 0
        for t in tl:
            lo_base[t] = b
            b += K_lo[t]
        b = skl
        for t in tl:
            hi_base[t] = b
            b += K_hi[t]
        ginfo.append(dict(tiles=tl, io=gio, dl=gdl, skl=skl, skh=skh,
                          lo_base=lo_base, hi_base=hi_base))
        gio += 8 * (skl + skh)
        gdl += skl + skh
    IDX_COLS = gio
    SUM_KT = gdl

    idx16 = np.zeros((NCORES, 128, IDX_COLS), np.int16)
    f8 = ml_dtypes.float8_e4m3
    ind = np.zeros((NCORES, 128, SUM_KT * 128), f8)
    indT = np.zeros((NCORES, 128, SUM_KT * 128), f8)
    one8 = f8(1.0)

    for c in range(NCORES):
        for gi in ginfo:
            tl, io, dl = gi["tiles"], gi["io"], gi["dl"]
            skl, skh = gi["skl"], gi["skh"]
            # --- src gather indices: lo block then hi block
            flat_lo = np.zeros(skl * P, np.int64)
            flat_hi = np.zeros(skh * P, np.int64)
            for t in tl:
                s_lo, d_lo = lo_lists[c][t]
                s_hi, d_hi = hi_lists[c][t]
                o = gi["lo_base"][t] * P
                flat_lo[o : o + len(s_lo)] = s_lo
                o = (gi["hi_base"][t] - skl) * P
                flat_hi[o : o + len(s_hi)] = s_hi
                # --- one-hot indicators
                i_lo = gi["lo_base"][t] * P + np.arange(len(d_lo))
                i_hi = gi["hi_base"][t] * P + np.arange(len(d_hi))
                ii = np.concatenate([i_lo, i_hi])
                dd = np.concatenate([d_lo, d_hi])
                pp = ii % P
                kk = ii // P
                base = (dl + kk) * 128
                ind[c, pp, base + dd] = one8
                indT[c, dd, base + pp] = one8
            idx16[c, :, io : io + 8 * skl] = _wrap_idx(flat_lo)
            idx16[c, :, io + 8 * skl : io + 8 * (skl + skh)] = _wrap_idx(flat_hi)

    return dict(
        K_lo=K_lo,
        K_hi=K_hi,
        ginfo=ginfo,
        IDX_COLS=IDX_COLS,
        SUM_KT=SUM_KT,
        idx16=idx16,
        ind=ind,
        indT=indT,
    )


# ---------------------------------------------------------------------------
# Bass program
# ---------------------------------------------------------------------------
def _build_program(ep):
    K_lo, K_hi, ginfo = ep["K_lo"], ep["K_hi"], ep["ginfo"]
    IDX_COLS, SUM_KT = ep["IDX_COLS"], ep["SUM_KT"]

    nc = bacc.Bacc("TRN2", target_bir_lowering=False, debug=False,
                   num_devices=NCORES)

    featsT_in = nc.dram_tensor("featsT", [IN, N_PAD], BF16, kind="ExternalInput")
    ownxT_in = nc.dram_tensor("ownxT", [IN, NPC], BF16, kind="ExternalInput")
    wcat0_in = nc.dram_tensor("wcat0", [IN, DCOLS], BF16, kind="ExternalInput")
    wcat1_in = nc.dram_tensor("wcat1", [HID, DCOLS], BF16, kind="ExternalInput")
    wres0_in = nc.dram_tensor("wres0", [IN, 4 + HID], BF16, kind="ExternalInput")
    wres1_in = nc.dram_tensor("wres1", [HID, 4 + HID], BF16, kind="ExternalInput")
    predw_in = nc.dram_tensor("predw", [HID, OUT], BF16, kind="ExternalInput")
    aux_in = nc.dram_tensor("aux", [P, 8 * 64], F32, kind="ExternalInput")
    ident_in = nc.dram_tensor("ident", [P, P], BF16, kind="ExternalInput")
    idx_in = nc.dram_tensor("idx16", [P, IDX_COLS], I16, kind="ExternalInput")
    ind_in = nc.dram_tensor("ind", [P, SUM_KT * 128], FP8, kind="ExternalInput")
    indT_in = nc.dram_tensor("indT", [P, SUM_KT * 128], FP8, kind="ExternalInput")
    out_t = nc.dram_tensor("out", [NPC, OUT], F32, kind="ExternalOutput")

    with tile.TileContext(nc) as tc:
        with (
            tc.tile_pool(name="const", bufs=1) as constp,
            tc.tile_pool(name="persist", bufs=1) as persist,
            tc.tile_pool(name="dense", bufs=4) as densep,
            tc.tile_pool(name="edge", bufs=2) as edgep,
            tc.tile_pool(name="epi", bufs=2) as epip,
            tc.tile_pool(name="dps", bufs=4, space="PSUM") as dps,
            tc.tile_pool(name="eps", bufs=2, space="PSUM") as eps,
            tc.tile_pool(name="aps", bufs=2, space="PSUM") as aps,
            tc.tile_pool(name="dram", bufs=1, space="DRAM") as dram,
        ):
            # ---- constants / persistent data
            wcat0 = constp.tile([IN, DCOLS], BF16)
            nc.sync.dma_start(out=wcat0[:], in_=wcat0_in[:, :])
            wcat1 = constp.tile([HID, DCOLS], BF16)
            nc.sync.dma_start(out=wcat1[:], in_=wcat1_in[:, :])
            wres0 = constp.tile([IN, 4 + HID], BF16)
            nc.sync.dma_start(out=wres0[:], in_=wres0_in[:, :])
            wres1 = constp.tile([HID, 4 + HID], BF16)
            nc.sync.dma_start(out=wres1[:], in_=wres1_in[:, :])
            predw = constp.tile([HID, OUT], BF16)
            nc.sync.dma_start(out=predw[:], in_=predw_in[:, :])
            aux = constp.tile([P, 8 * 64], F32)
            nc.sync.dma_start(out=aux[:], in_=aux_in[:, :])
            ident = constp.tile([P, P], BF16)
            nc.sync.dma_start(out=ident[:], in_=ident_in[:, :])
            idx16 = persist.tile([P, IDX_COLS], I16)
            nc.sync.dma_start(out=idx16[:], in_=idx_in[:, :])

            gml = [aux[:, 0:64], aux[:, 128:192]]
            bml = [aux[:, 64:128], aux[:, 192:256]]
            resbl = [aux[:, 256:320], aux[:, 320:384]]
            predb = aux[:, 384:448]
            eps_col = aux[:, 448:449]

            dcat = [
                dram.tile([N_PAD, RL], BF16, name="dcat0", tag="dcat0"),
                dram.tile([N_PAD, RL], BF16, name="dcat1", tag="dcat1"),
            ]
            h1ownT = dram.tile([HID, NPC], BF16)
            h1Tb = dram.tile([NCORES, HID, NPC], BF16, addr_space="Shared")

            # own-rows er (bf16, for the er-broadcast matmul) and res (f32)
            er_sb = persist.tile([P, T, 4], BF16)
            res_sb = persist.tile([P, T, HID], F32)

            # =============== phases ===============
            def dense_group(li, dc, wcat, din, tlist, src_ap):
                nt = len(tlist)
                xTg = densep.tile([din, GD, P], BF16, tag="xT")
                nc.sync.dma_start(out=xTg[:, 0:nt, :], in_=src_ap)
                dcps = [
                    dps.tile([P, DCOLS], F32, tag="dc_ps", name=f"dc_ps_{i}")
                    for i in range(nt)
                ]
                for i in range(nt):
                    nc.tensor.matmul(
                        out=dcps[i][:], lhsT=xTg[:, i, :], rhs=wcat[:],
                        start=True, stop=True,
                    )
                dcb = densep.tile([P, GD, DCOLS], BF16, tag="dcb")
                for i in range(nt):
                    nc.scalar.copy(out=dcb[:, i, :], in_=dcps[i][:])
                gt0 = tlist[0]
                out_ap = dc[gt0 * P : (gt0 + nt) * P, 0:DCOLS].rearrange(
                    "(i p) c -> p i c", p=P
                )
                nc.sync.dma_start(out=out_ap, in_=dcb[:, 0:nt, :])

            def dense_phase(li):
                din = IN if li == 0 else HID
                wcat = wcat0 if li == 0 else wcat1
                dc = dcat[li]
                if li == 0:
                    for g0 in range(0, TILES_G, GD):
                        tlist = list(range(g0, min(g0 + GD, TILES_G)))
                        src = featsT_in[:, g0 * P : (g0 + len(tlist)) * P]
                        dense_group(li, dc, wcat, din, tlist, src)
                else:
                    for cblk in range(NCORES):
                        for t0 in range(0, T, GD):
                            tlist = [cblk * T + t for t in
                                     range(t0, min(t0 + GD, T))]
                            src = h1Tb[
                                cblk, :, t0 * P : (t0 + len(tlist)) * P
                            ]
                            dense_group(li, dc, wcat, din, tlist, src)

            def own_phase(li):
                din = IN if li == 0 else HID
                wres = wres0 if li == 0 else wres1
                xsrc = ownxT_in if li == 0 else h1ownT
                for t0 in range(0, T, GD):
                    nt = min(GD, T - t0)
                    oxT = densep.tile([din, GD, P], BF16, tag="oxT")
                    nc.sync.dma_start(
                        out=oxT[:, 0:nt, :],
                        in_=xsrc[:, t0 * P : (t0 + nt) * P],
                    )
                    for i in range(nt):
                        own_ps = dps.tile([P, 4 + HID], F32, tag="dc_ps")
                        nc.tensor.matmul(
                            out=own_ps[:], lhsT=oxT[:, i, :], rhs=wres[:],
                            start=True, stop=True,
                        )
                        nc.scalar.copy(
                            out=er_sb[:, t0 + i, :], in_=own_ps[:, 0:4]
                        )
                        nc.scalar.copy(
                            out=res_sb[:, t0 + i, :], in_=own_ps[:, 4 : 4 + HID]
                        )

            def edge_phase(li):
                dc = dcat[li]
                g_ln, b_ln, resb = gml[li], bml[li], resbl[li]
                for gi in ginfo:
                    tl, io, dl = gi["tiles"], gi["io"], gi["dl"]
                    skl, skh = gi["skl"], gi["skh"]
                    ktq = skl + skh
                    g = edgep.tile([P, ktq, RL], BF16, tag="gbuf")
                    nc.gpsimd.dma_gather(
                        out_ap=g[:, 0:skl, :],
                        in_ap=dc[0:NHALF, :],
                        idxs_ap=idx16[:, io : io + 8 * skl],
                        num_idxs=skl * P,
                        num_idxs_reg=skl * P,
                        elem_size=RL,
                        elem_step=RL,
                        single_packet=(skl * P <= 1024),
                    )
                    nc.gpsimd.dma_gather(
                        out_ap=g[:, skl:ktq, :],
                        in_ap=dc[NHALF:N_PAD, :],
                        idxs_ap=idx16[:, io + 8 * skl : io + 8 * ktq],
                        num_idxs=skh * P,
                        num_idxs_reg=skh * P,
                        elem_size=RL,
                        elem_step=RL,
                        single_packet=(skh * P <= 1024),
                    )
                    co = dl * 128
                    indb = edgep.tile([P, ktq, P], FP8, tag="indb")
                    nc.sync.dma_start(
                        out=indb[:], in_=ind_in[:, co : co + ktq * 128]
                    )
                    indTb = edgep.tile([P, ktq, P], FP8, tag="indTb")
                    nc.sync.dma_start(
                        out=indTb[:], in_=indT_in[:, co : co + ktq * 128]
                    )
                    # er[dst] per slot: er_ps[s, :] = sum_n indT[n, s] er[n, :]
                    chunk_tile = {}
                    for t in tl:
                        for k in range(K_lo[t]):
                            chunk_tile[gi["lo_base"][t] + k] = t
                        for k in range(K_hi[t]):
                            chunk_tile[gi["hi_base"][t] + k] = t
                    er_ps = eps.tile([P, ktq, 4], F32, tag="er_ps")
                    for c in range(ktq):
                        nc.tensor.matmul(
                            out=er_ps[:, c, :],
                            lhsT=indTb[:, c, :],
                            rhs=er_sb[:, chunk_tile[c] % T, :],
                            start=True, stop=True,
                        )
                    # p = exp(leakyrelu(el_src + er_dst))
                    xb = edgep.tile([P, ktq, 4], F32, tag="xb")
                    nc.vector.tensor_tensor(
                        out=xb[:], in0=g[:, :, C_EL : C_EL + 4], in1=er_ps[:],
                        op=OP.add,
                    )
                    x2 = edgep.tile([P, ktq, 4], F32, tag="x2")
                    nc.vector.scalar_tensor_tensor(
                        out=x2[:], in0=xb[:], scalar=NEG_SLOPE, in1=xb[:],
                        op0=OP.mult, op1=OP.max,
                    )
                    mbt = edgep.tile([P, ktq, DCOLS], BF16, tag="mbt")
                    nc.scalar.activation(
                        out=mbt[:, :, 256:260], in_=x2[:], func=AF.Exp
                    )
                    # msg = p * fs[src] (broadcast p over the 64 cols per head)
                    msg4 = mbt[:, :, 0:256].rearrange(
                        "p k (h f) -> p k h f", f=64
                    )
                    nc.vector.tensor_tensor(
                        out=msg4,
                        in0=g[:, :, 0:256].rearrange(
                            "p k (h f) -> p k h f", f=64
                        ),
                        in1=mbt[:, :, 256:260].to_broadcast([P, ktq, 4, 64]),
                        op=OP.mult,
                    )
                    for t in tl:
                        chunks = (
                            list(range(gi["lo_base"][t],
                                       gi["lo_base"][t] + K_lo[t]))
                            + list(range(gi["hi_base"][t],
                                         gi["hi_base"][t] + K_hi[t]))
                        )
                        # scatter-add into PSUM by dst
                        ps = aps.tile([P, DCOLS], F32, tag="agg")
                        for j, c in enumerate(chunks):
                            nc.tensor.matmul(
                                out=ps[:],
                                lhsT=indb[:, c, :],
                                rhs=mbt[:, c, :],
                                start=(j == 0),
                                stop=(j == len(chunks) - 1),
                            )
                        # ---- fused epilogue for this tile
                        S = ps[:, 0:256]
                        asum = ps[:, 256:260]
                        r4 = epip.tile([P, 4], F32, tag="r4")
                        nc.vector.tensor_scalar(
                            r4[:], asum, 4.0, TINY, OP.mult, OP.max
                        )
                        rec = epip.tile([P, 4], F32, tag="rec")
                        nc.vector.reciprocal(rec[:], r4[:])
                        m = epip.tile([P, HID], F32, tag="m")
                        nc.vector.tensor_scalar(
                            m[:], S[:, 0:64], rec[:, 0:1], None, OP.mult
                        )
                        for h in range(1, H):
                            nc.vector.scalar_tensor_tensor(
                                out=m[:],
                                in0=S[:, 64 * h : 64 * (h + 1)],
                                scalar=rec[:, h : h + 1],
                                in1=m[:],
                                op0=OP.mult,
                                op1=OP.add,
                            )
                        xr = epip.tile([P, HID], F32, tag="xr")
                        nc.vector.tensor_tensor(
                            out=xr[:], in0=m[:], in1=res_sb[:, t, :], op=OP.add
                        )
                        nc.vector.tensor_tensor(
                            out=xr[:], in0=xr[:], in1=resb, op=OP.add
                        )
                        # LayerNorm + ReLU
                        stat = epip.tile([P, 8], F32, tag="stat")
                        nc.vector.tensor_reduce(
                            out=stat[:, 0:1], in_=xr[:], axis=AX.X, op=OP.add
                        )
                        nc.vector.tensor_scalar_mul(
                            stat[:, 1:2], stat[:, 0:1], 1.0 / HID
                        )
                        xc = epip.tile([P, HID], F32, tag="xc")
                        nc.vector.tensor_scalar(
                            xc[:], xr[:], stat[:, 1:2], None, OP.subtract
                        )
                        sq = epip.tile([P, HID], F32, tag="sq")
                        nc.scalar.activation(
                            out=sq[:], in_=xc[:], func=AF.Square,
                            accum_out=stat[:, 2:3],
                        )
                        nc.scalar.activation(
                            out=stat[:, 3:4], in_=stat[:, 2:3], func=AF.Sqrt,
                            bias=eps_col, scale=1.0 / HID,
                        )
                        nc.vector.reciprocal(stat[:, 4:5], stat[:, 3:4])
                        y = epip.tile([P, HID], F32, tag="y")
                        nc.vector.scalar_tensor_tensor(
                            out=y[:], in0=xc[:], scalar=stat[:, 4:5], in1=g_ln,
                            op0=OP.mult, op1=OP.mult,
                        )
                        ht = epip.tile([P, HID], BF16, tag="ht")
                        nc.vector.tensor_tensor(
                            out=ht[:], in0=y[:], in1=b_ln, op=OP.add
                        )
                        nc.vector.tensor_scalar_max(ht[:], ht[:], 0.0)
                        hT_ps = aps.tile([HID, P], BF16, tag="agg")
                        nc.tensor.transpose(
                            out=hT_ps[:], in_=ht[:], identity=ident[:]
                        )
                        hT = epip.tile([HID, P], BF16, tag="hT")
                        nc.scalar.copy(out=hT[:], in_=hT_ps[:])
                        if li == 0:
                            nc.sync.dma_start(
                                out=h1ownT[:, t * P : (t + 1) * P], in_=hT[:]
                            )
                        else:
                            hd_ps = dps.tile([P, OUT], F32, tag="dc_ps")
                            nc.tensor.matmul(
                                out=hd_ps[:], lhsT=hT[:], rhs=predw[:],
                                start=True, stop=True,
                            )
                            ob = epip.tile([P, OUT], F32, tag="ob")
                            nc.vector.tensor_tensor(
                                out=ob[:], in0=hd_ps[:], in1=predb, op=OP.add
                            )
                            nc.sync.dma_start(
                                out=out_t[t * P : (t + 1) * P, :], in_=ob[:]
                            )

            # =============== schedule ===============
            stages = os.environ.get(
                "GAT_STAGES", "d0,o0,e0,c,d1,o1,e1"
            ).split(",")
            if "d0" in stages:
                dense_phase(0)
            if "o0" in stages:
                own_phase(0)
            if "e0" in stages:
                edge_phase(0)
            if "c" in stages:
                nc.gpsimd.collective_compute(
                    "AllGather",
                    OP.bypass,
                    replica_groups=[list(range(NCORES))],
                    ins=[h1ownT[:].opt()],
                    outs=[h1Tb[:].opt()],
                )
            if "d1" in stages:
                dense_phase(1)
            if "o1" in stages:
                own_phase(1)
            if "e1" in stages:
                edge_phase(1)
            if "e1" not in stages:
                # keep the output tensor written so PJRT returns
                dummy = epip.tile([P, OUT], F32, tag="ob")
                nc.vector.memset(dummy[:], 0.0)
                nc.sync.dma_start(out=out_t[0:P, :], in_=dummy[:])

    nc.compile()
    return nc


# ---------------------------------------------------------------------------
# Host entry point
# ---------------------------------------------------------------------------
def kernel(feats, src, dst, W0, al0, ar0, resw0, resb0, g0, b0,
           W1, al1, ar1, resw1, resb1, g1, b1, predw, predb):
    f32 = np.float32
    bf16 = ml_dtypes.bfloat16
    feats = np.asarray(feats, f32)
    W0 = np.asarray(W0, f32)
    al0 = np.asarray(al0, f32)
    ar0 = np.asarray(ar0, f32)
    resw0 = np.asarray(resw0, f32)
    W1 = np.asarray(W1, f32)
    al1 = np.asarray(al1, f32)
    ar1 = np.asarray(ar1, f32)
    resw1 = np.asarray(resw1, f32)
    predw_np = np.asarray(predw, f32)

    ep = _prep_edges(src, dst)
    nc = _build_program(ep)

    feats_pad = np.zeros((N_PAD, IN), f32)
    feats_pad[:N] = feats
    featsT = np.ascontiguousarray(feats_pad.T).astype(bf16)

    def fold(W, a):
        return (W.reshape(W.shape[0], H, HID) * a[None]).sum(-1)

    wcat0 = np.concatenate([W0, fold(W0, al0)], axis=1)
    wcat1 = np.concatenate([W1, fold(W1, al1)], axis=1)
    wres0 = np.concatenate([fold(W0, ar0), resw0], axis=1)
    wres1 = np.concatenate([fold(W1, ar1), resw1], axis=1)

    aux = np.zeros((P, 8 * 64), f32)
    aux[:, 0:64] = np.asarray(g0, f32)[None]
    aux[:, 64:128] = np.asarray(b0, f32)[None]
    aux[:, 128:192] = np.asarray(g1, f32)[None]
    aux[:, 192:256] = np.asarray(b1, f32)[None]
    aux[:, 256:320] = np.asarray(resb0, f32)[None]
    aux[:, 320:384] = np.asarray(resb1, f32)[None]
    aux[:, 384:448] = np.asarray(predb, f32)[None]
    aux[:, 448] = EPS

    ident = np.eye(P, dtype=f32).astype(bf16)

    shared = {
        "featsT": featsT,
        "wcat0": np.ascontiguousarray(wcat0).astype(bf16),
        "wcat1": np.ascontiguousarray(wcat1).astype(bf16),
        "wres0": np.ascontiguousarray(wres0).astype(bf16),
        "wres1": np.ascontiguousarray(wres1).astype(bf16),
        "predw": np.ascontiguousarray(predw_np).astype(bf16),
        "aux": aux,
        "ident": ident,
    }
    in_maps = [
        {
            **shared,
            "ownxT": np.ascontiguousarray(featsT[:, c * NPC : (c + 1) * NPC]),
            "idx16": ep["idx16"][c],
            "ind": ep["ind"][c],
            "indT": ep["indT"][c],
        }
        for c in range(NCORES)
    ]

    trace = os.environ.get("GAT_TRACE", "0") == "1"
    res = run_bass_kernel_spmd(
        nc, in_maps, core_ids=list(range(NCORES)), trace=trace
    )
    if trace and res.exec_time_ns is not None:
        print(f"HW exec time: {res.exec_time_ns} ns")
        if res.instructions_and_trace is not None:
            print(f"trace: {res.instructions_and_trace[1]}")

    out = np.concatenate([res.results[c]["out"] for c in range(NCORES)], axis=0)
    return np.ascontiguousarray(out[:N]).astype(np.float32)


# revision 12
# speedup vs baseline: 1.3733x; 1.3733x over previous
"""2-layer 4-head GAT (DGL GATConv-style) as a distributed Bass/Tile kernel
on 8 Trainium2 NeuronCores.

Sharding: destination nodes split 6272/core (49 tiles of 128). Per layer every
core redundantly computes the dense projections for ALL nodes into a bf16
"dcat" table in its HBM (4 node-tiles per matmul group): row n =
[X@W (256) | X@w_el (4) | pad] at a 768B stride (w_el is host-prefolded W@al).
The edge phase then, per GROUP of two 128-destination tiles, dma_gathers the
[fs|el] rows by src (the ONLY per-edge gathers; two calls per group, one per
int16-indexable table half), and:
  - er[dst] per edge slot comes from a tiny per-chunk matmul with a
    host-streamed transposed one-hot indicator (indT, fp8) against the tile's
    er column (from the own-rows phase) -- no dst-side gather.
  - p = exp(leakyrelu(el_src + er_dst)); messages scaled by p on DVE;
  - scatter-add into PSUM [128, 260] via host-streamed one-hot indicator
    matmuls (ind, fp8) -- no on-chip indicator builds.
The per-tile epilogue (fused) normalizes by the attention sums, means heads,
adds the residual (own-rows phase) and LayerNorm+ReLU, then stores the tile
TRANSPOSED (h1ownT, bf16); layer-0 results are AllGathered as h1T blocks so
layer-1's dense phase needs no transposes; the layer-1 epilogue applies the
prediction head.

The "own-rows" phase computes er/res for the core's own 6272 nodes from
per-core pre-transposed inputs (ownxT for layer 0, h1ownT for layer 1), so no
per-core addressing of shared tables is needed anywhere.
"""
import contextlib
import ctypes
import os
import sys
import types

import numpy as np

sys.path.insert(0, "/opt/trn_rl_repo")

import ml_dtypes  # noqa: E402

# ---------------------------------------------------------------------------
# Shim: antenv.axon_hooks (missing in this image) so trace=True works.
# ---------------------------------------------------------------------------
_ntff_hook = None


def _install_axon_hooks_shim():
    global _ntff_hook
    if "antenv.axon_hooks" in sys.modules:
        return
    try:
        import antenv
    except ImportError:
        return
    mod = types.ModuleType("antenv.axon_hooks")

    def set_axon_ntff_profile_hook(h):
        global _ntff_hook
        _ntff_hook = h

    def get_axon_ntff_profile_hook():
        return _ntff_hook

    mod.set_axon_ntff_profile_hook = set_axon_ntff_profile_hook
    mod.get_axon_ntff_profile_hook = get_axon_ntff_profile_hook
    sys.modules["antenv.axon_hooks"] = mod
    antenv.axon_hooks = mod

    so_path = "/opt/axon/libaxon_pjrt.so"
    try:
        lib = ctypes.CDLL(so_path)
    except OSError:
        return
    if not hasattr(lib, "axon_start_nrt_profile"):
        return
    lib.axon_start_nrt_profile.argtypes = [
        ctypes.POINTER(ctypes.c_int64),
        ctypes.c_size_t,
    ]
    lib.axon_start_nrt_profile.restype = ctypes.c_int64
    lib.axon_stop_nrt_profile.argtypes = [ctypes.c_char_p]
    lib.axon_stop_nrt_profile.restype = ctypes.c_int64

    @contextlib.contextmanager
    def _hook(output_dir, device_ids):
        import jax

        jax.devices()
        if device_ids:
            ids = (ctypes.c_int64 * len(device_ids))(*device_ids)
            rc = lib.axon_start_nrt_profile(ids, len(device_ids))
        else:
            rc = lib.axon_start_nrt_profile(None, 0)
        if rc != 0:
            raise RuntimeError(f"axon_start_nrt_profile rc={rc}")
        try:
            yield
        finally:
            n = lib.axon_stop_nrt_profile(str(output_dir).encode())
            if n < 0:
                raise RuntimeError(f"axon_stop_nrt_profile rc={n}")
            print(f"profile: {n} file(s) written to {output_dir}", file=sys.stderr)

    set_axon_ntff_profile_hook(_hook)


_install_axon_hooks_shim()

import concourse.bass as bass  # noqa: E402
import concourse.bacc as bacc  # noqa: E402
import concourse.mybir as mybir  # noqa: E402
import concourse.tile as tile  # noqa: E402
from concourse.bass_utils import run_bass_kernel_spmd  # noqa: E402


# ---------------------------------------------------------------------------
# Problem constants (kernel.py is self-contained; shapes are hardcoded).
# ---------------------------------------------------------------------------
N, E = 50000, 800000
IN, HID, H, OUT = 128, 64, 4, 64
NEG_SLOPE = 0.2
EPS = 1e-5

P = 128
NCORES = 8
T = 49                       # dst node tiles per core
NPC = T * P                  # 6272 nodes per core
N_PAD = NCORES * NPC         # 50176 (node space)
NHALF = N_PAD // 2           # 25088 rows per dcat half (int16-indexable)
RL = 384                     # dcat row length (bf16) -> 768B stride
DCOLS = 260                  # occupied dcat cols: [fs 256 | el 4]
C_EL = 256
TINY = 1e-30
TILES_G = N_PAD // P         # 392 global tiles
GD = 4                       # dense-phase tiles per matmul group

# edge-phase tile groups (gathers/streams merged per group)
GROUPS = [[t, t + 1] for t in range(0, T - 1, 2)] + [[T - 1]]

F32 = mybir.dt.float32
BF16 = mybir.dt.bfloat16
FP8 = mybir.dt.float8e4
I16 = mybir.dt.int16
OP = mybir.AluOpType
AF = mybir.ActivationFunctionType
AX = mybir.AxisListType


def _cdiv(a, b):
    return (a + b - 1) // b


# ---------------------------------------------------------------------------
# Host-side edge preprocessing
# ---------------------------------------------------------------------------
def _wrap_idx(flat):
    """dma_gather index layout: idx j -> [j%16, j//16], replicated to 128
    partitions."""
    n = len(flat)
    assert n % 128 == 0
    cols = n // 16
    w = np.zeros((16, cols), np.int16)
    w[np.arange(n) % 16, np.arange(n) // 16] = flat
    return np.tile(w, (8, 1))


def _prep_edges(src, dst):
    src = np.asarray(src).astype(np.int64)
    dst = np.asarray(dst).astype(np.int64)
    order = np.argsort(dst, kind="stable")
    src = src[order]
    dst = dst[order]

    bounds = np.searchsorted(dst, np.arange(0, N_PAD + 1, P))

    lo_lists = [[None] * T for _ in range(NCORES)]
    hi_lists = [[None] * T for _ in range(NCORES)]
    for c in range(NCORES):
        for t in range(T):
            gt = c * T + t
            e0, e1 = bounds[gt], bounds[gt + 1]
            s = np.asarray(src[e0:e1])
            d = np.asarray(dst[e0:e1]) - gt * P
            is_lo = s < NHALF
            lo_lists[c][t] = (s[is_lo], d[is_lo])
            hi_lists[c][t] = (s[~is_lo] - NHALF, d[~is_lo])

    K_lo = [
        max(1, max(_cdiv(len(lo_lists[c][t][0]), P) for c in range(NCORES)))
        for t in range(T)
    ]
    K_hi = [
        max(1, max(_cdiv(len(hi_lists[c][t][0]), P) for c in range(NCORES)))
        for t in range(T)
    ]

    # group metadata: per group the idx col offset, chunk offset, and per-tile
    # chunk bases (lo chunks of all tiles first, then hi chunks).
    ginfo = []
    gio = 0
    gdl = 0
    for tl in GROUPS:
        skl = sum(K_lo[t] for t in tl)
        skh = sum(K_hi[t] for t in tl)
        lo_base = {}
        hi_base = {}
        b = 0
        for t in tl:
            lo_base[t] = b
            b += K_lo[t]
        b = skl
        for t in tl:
            hi_base[t] = b
            b += K_hi[t]
        ginfo.append(dict(tiles=tl, io=gio, dl=gdl, skl=skl, skh=skh,
                          lo_base=lo_base, hi_base=hi_base))
        gio += 8 * (skl + skh)
        gdl += skl + skh
    IDX_COLS = gio
    SUM_KT = gdl

    idx16 = np.zeros((NCORES, 128, IDX_COLS), np.int16)
    f8 = ml_dtypes.float8_e4m3
    ind = np.zeros((NCORES, 128, SUM_KT * 128), f8)
    indT = np.zeros((NCORES, 128, SUM_KT * 128), f8)
    one8 = f8(1.0)

    for c in range(NCORES):
        for gi in ginfo:
            tl, io, dl = gi["tiles"], gi["io"], gi["dl"]
            skl, skh = gi["skl"], gi["skh"]
            # --- src gather indices: lo block then hi block
            flat_lo = np.zeros(skl * P, np.int64)
            flat_hi = np.zeros(skh * P, np.int64)
            for t in tl:
                s_lo, d_lo = lo_lists[c][t]
                s_hi, d_hi = hi_lists[c][t]
                o = gi["lo_base"][t] * P
                flat_lo[o : o + len(s_lo)] = s_lo
                o = (gi["hi_base"][t] - skl) * P
                flat_hi[o : o + len(s_hi)] = s_hi
                # --- one-hot indicators
                i_lo = gi["lo_base"][t] * P + np.arange(len(d_lo))
                i_hi = gi["hi_base"][t] * P + np.arange(len(d_hi))
                ii = np.concatenate([i_lo, i_hi])
                dd = np.concatenate([d_lo, d_hi])
                pp = ii % P
                kk = ii // P
                base = (dl + kk) * 128
                ind[c, pp, base + dd] = one8
                indT[c, dd, base + pp] = one8
            idx16[c, :, io : io + 8 * skl] = _wrap_idx(flat_lo)
            idx16[c, :, io + 8 * skl : io + 8 * (skl + skh)] = _wrap_idx(flat_hi)

    return dict(
        K_lo=K_lo,
        K_hi=K_hi,
        ginfo=ginfo,
        IDX_COLS=IDX_COLS,
        SUM_KT=SUM_KT,
        idx16=idx16,
        ind=ind,
        indT=indT,
    )


# ---------------------------------------------------------------------------
# Bass program
# ---------------------------------------------------------------------------
def _build_program(ep):
    K_lo, K_hi, ginfo = ep["K_lo"], ep["K_hi"], ep["ginfo"]
    IDX_COLS, SUM_KT = ep["IDX_COLS"], ep["SUM_KT"]

    nc = bacc.Bacc("TRN2", target_bir_lowering=False, debug=False,
                   num_devices=NCORES)

    featsT_in = nc.dram_tensor("featsT", [IN, N_PAD], BF16, kind="ExternalInput")
    ownxT_in = nc.dram_tensor("ownxT", [IN, NPC], BF16, kind="ExternalInput")
    wcat0_in = nc.dram_tensor("wcat0", [IN, DCOLS], BF16, kind="ExternalInput")
    wcat1_in = nc.dram_tensor("wcat1", [HID, DCOLS], BF16, kind="ExternalInput")
    wres0_in = nc.dram_tensor("wres0", [IN, 4 + HID], BF16, kind="ExternalInput")
    wres1_in = nc.dram_tensor("wres1", [HID, 4 + HID], BF16, kind="ExternalInput")
    predw_in = nc.dram_tensor("predw", [HID, OUT], BF16, kind="ExternalInput")
    aux_in = nc.dram_tensor("aux", [P, 8 * 64], F32, kind="ExternalInput")
    ident_in = nc.dram_tensor("ident", [P, P], BF16, kind="ExternalInput")
    idx_in = nc.dram_tensor("idx16", [P, IDX_COLS], I16, kind="ExternalInput")
    ind_in = nc.dram_tensor("ind", [P, SUM_KT * 128], FP8, kind="ExternalInput")
    indT_in = nc.dram_tensor("indT", [P, SUM_KT * 128], FP8, kind="ExternalInput")
    out_t = nc.dram_tensor("out", [NPC, OUT], F32, kind="ExternalOutput")

    with tile.TileContext(nc) as tc:
        with (
            tc.tile_pool(name="const", bufs=1) as constp,
            tc.tile_pool(name="persist", bufs=1) as persist,
            tc.tile_pool(name="dense", bufs=3) as densep,
            tc.tile_pool(name="gpool", bufs=3) as gpool,
            tc.tile_pool(name="edge", bufs=2) as edgep,
            tc.tile_pool(name="epi", bufs=2) as epip,
            tc.tile_pool(name="dps", bufs=2, space="PSUM") as dps,
            tc.tile_pool(name="eps", bufs=2, space="PSUM") as eps,
            tc.tile_pool(name="aps", bufs=2, space="PSUM") as aps,
            tc.tile_pool(name="tps", bufs=2, space="PSUM") as tps,
            tc.tile_pool(name="dram", bufs=1, space="DRAM") as dram,
        ):
            # ---- constants / persistent data
            wcat0 = constp.tile([IN, DCOLS], BF16)
            nc.sync.dma_start(out=wcat0[:], in_=wcat0_in[:, :])
            wcat1 = constp.tile([HID, DCOLS], BF16)
            nc.sync.dma_start(out=wcat1[:], in_=wcat1_in[:, :])
            wres0 = constp.tile([IN, 4 + HID], BF16)
            nc.sync.dma_start(out=wres0[:], in_=wres0_in[:, :])
            wres1 = constp.tile([HID, 4 + HID], BF16)
            nc.sync.dma_start(out=wres1[:], in_=wres1_in[:, :])
            predw = constp.tile([HID, OUT], BF16)
            nc.sync.dma_start(out=predw[:], in_=predw_in[:, :])
            aux = constp.tile([P, 8 * 64], F32)
            nc.sync.dma_start(out=aux[:], in_=aux_in[:, :])
            ident = constp.tile([P, P], BF16)
            nc.sync.dma_start(out=ident[:], in_=ident_in[:, :])
            idx16 = persist.tile([P, IDX_COLS], I16)
            nc.sync.dma_start(out=idx16[:], in_=idx_in[:, :])

            gml = [aux[:, 0:64], aux[:, 128:192]]
            bml = [aux[:, 64:128], aux[:, 192:256]]
            resbl = [aux[:, 256:320], aux[:, 320:384]]
            predb = aux[:, 384:448]
            eps_col = aux[:, 448:449]

            dcat = [
                dram.tile([N_PAD, RL], BF16, name="dcat0", tag="dcat0"),
                dram.tile([N_PAD, RL], BF16, name="dcat1", tag="dcat1"),
            ]
            h1ownT = dram.tile([HID, NPC], BF16)
            h1Tb = dram.tile([NCORES, HID, NPC], BF16, addr_space="Shared")

            # own-rows er (bf16, for the er-broadcast matmul) and res (f32)
            er_sb = persist.tile([P, T, 4], BF16)
            res_sb = persist.tile([P, T, HID], F32)

            # =============== phases ===============
            def dense_group(li, dc, wcat, din, tlist, src_ap):
                nt = len(tlist)
                xTg = densep.tile([din, GD, P], BF16, tag="xT")
                nc.sync.dma_start(out=xTg[:, 0:nt, :], in_=src_ap)
                dcps = [
                    dps.tile([P, DCOLS], F32, tag="dc_ps", name=f"dc_ps_{i}")
                    for i in range(nt)
                ]
                for i in range(nt):
                    nc.tensor.matmul(
                        out=dcps[i][:], lhsT=xTg[:, i, :], rhs=wcat[:],
                        start=True, stop=True,
                    )
                dcb = densep.tile([P, GD, DCOLS], BF16, tag="dcb")
                for i in range(nt):
                    nc.vector.tensor_copy(out=dcb[:, i, :], in_=dcps[i][:])
                gt0 = tlist[0]
                out_ap = dc[gt0 * P : (gt0 + nt) * P, 0:DCOLS].rearrange(
                    "(i p) c -> p i c", p=P
                )
                nc.sync.dma_start(out=out_ap, in_=dcb[:, 0:nt, :])

            def dense_phase(li):
                din = IN if li == 0 else HID
                wcat = wcat0 if li == 0 else wcat1
                dc = dcat[li]
                if li == 0:
                    for g0 in range(0, TILES_G, GD):
                        tlist = list(range(g0, min(g0 + GD, TILES_G)))
                        src = featsT_in[:, g0 * P : (g0 + len(tlist)) * P]
                        dense_group(li, dc, wcat, din, tlist, src)
                else:
                    for cblk in range(NCORES):
                        for t0 in range(0, T, GD):
                            tlist = [cblk * T + t for t in
                                     range(t0, min(t0 + GD, T))]
                            src = h1Tb[
                                cblk, :, t0 * P : (t0 + len(tlist)) * P
                            ]
                            dense_group(li, dc, wcat, din, tlist, src)

            def own_phase(li):
                din = IN if li == 0 else HID
                wres = wres0 if li == 0 else wres1
                xsrc = ownxT_in if li == 0 else h1ownT
                for t0 in range(0, T, GD):
                    nt = min(GD, T - t0)
                    oxT = densep.tile([din, GD, P], BF16, tag="oxT")
                    nc.sync.dma_start(
                        out=oxT[:, 0:nt, :],
                        in_=xsrc[:, t0 * P : (t0 + nt) * P],
                    )
                    for i in range(nt):
                        own_ps = dps.tile([P, 4 + HID], F32, tag="dc_ps")
                        nc.tensor.matmul(
                            out=own_ps[:], lhsT=oxT[:, i, :], rhs=wres[:],
                            start=True, stop=True,
                        )
                        nc.scalar.copy(
                            out=er_sb[:, t0 + i, :], in_=own_ps[:, 0:4]
                        )
                        nc.scalar.copy(
                            out=res_sb[:, t0 + i, :], in_=own_ps[:, 4 : 4 + HID]
                        )

            def edge_phase(li):
                dc = dcat[li]
                g_ln, b_ln, resb = gml[li], bml[li], resbl[li]
                for gi in ginfo:
                    tl, io, dl = gi["tiles"], gi["io"], gi["dl"]
                    skl, skh = gi["skl"], gi["skh"]
                    ktq = skl + skh
                    g = gpool.tile([P, ktq, RL], BF16, tag="gbuf")
                    nc.gpsimd.dma_gather(
                        out_ap=g[:, 0:skl, :],
                        in_ap=dc[0:NHALF, :],
                        idxs_ap=idx16[:, io : io + 8 * skl],
                        num_idxs=skl * P,
                        num_idxs_reg=skl * P,
                        elem_size=RL,
                        elem_step=RL,
                        single_packet=(skl * P <= 1024),
                    )
                    nc.gpsimd.dma_gather(
                        out_ap=g[:, skl:ktq, :],
                        in_ap=dc[NHALF:N_PAD, :],
                        idxs_ap=idx16[:, io + 8 * skl : io + 8 * ktq],
                        num_idxs=skh * P,
                        num_idxs_reg=skh * P,
                        elem_size=RL,
                        elem_step=RL,
                        single_packet=(skh * P <= 1024),
                    )
                    co = dl * 128
                    indb = edgep.tile([P, ktq, P], FP8, tag="indb")
                    nc.sync.dma_start(
                        out=indb[:], in_=ind_in[:, co : co + ktq * 128]
                    )
                    indTb = edgep.tile([P, ktq, P], FP8, tag="indTb")
                    nc.sync.dma_start(
                        out=indTb[:], in_=indT_in[:, co : co + ktq * 128]
                    )
                    # er[dst] per slot: er_ps[s, :] = sum_n indT[n, s] er[n, :]
                    chunk_tile = {}
                    for t in tl:
                        for k in range(K_lo[t]):
                            chunk_tile[gi["lo_base"][t] + k] = t
                        for k in range(K_hi[t]):
                            chunk_tile[gi["hi_base"][t] + k] = t
                    er_ps = eps.tile([P, ktq, 4], F32, tag="er_ps")
                    for c in range(ktq):
                        nc.tensor.matmul(
                            out=er_ps[:, c, :],
                            lhsT=indTb[:, c, :],
                            rhs=er_sb[:, chunk_tile[c] % T, :],
                            start=True, stop=True,
                        )
                    # p = exp(leakyrelu(el_src + er_dst))
                    xb = edgep.tile([P, ktq, 4], F32, tag="xb")
                    nc.vector.tensor_tensor(
                        out=xb[:], in0=g[:, :, C_EL : C_EL + 4], in1=er_ps[:],
                        op=OP.add,
                    )
                    x2 = edgep.tile([P, ktq, 4], F32, tag="x2")
                    nc.scalar.activation(
                        out=x2[:], in_=xb[:], func=AF.Lrelu, alpha=NEG_SLOPE
                    )
                    mbt = edgep.tile([P, ktq, DCOLS], BF16, tag="mbt")
                    nc.scalar.activation(
                        out=mbt[:, :, 256:260], in_=x2[:], func=AF.Exp
                    )
                    # msg = p * fs[src] (broadcast p over the 64 cols per head)
                    msg4 = mbt[:, :, 0:256].rearrange(
                        "p k (h f) -> p k h f", f=64
                    )
                    nc.vector.tensor_tensor(
                        out=msg4,
                        in0=g[:, :, 0:256].rearrange(
                            "p k (h f) -> p k h f", f=64
                        ),
                        in1=mbt[:, :, 256:260].to_broadcast([P, ktq, 4, 64]),
                        op=OP.mult,
                    )
                    for t in tl:
                        chunks = (
                            list(range(gi["lo_base"][t],
                                       gi["lo_base"][t] + K_lo[t]))
                            + list(range(gi["hi_base"][t],
                                         gi["hi_base"][t] + K_hi[t]))
                        )
                        # scatter-add into PSUM by dst
                        ps = aps.tile([P, DCOLS], F32, tag="agg")
                        for j, c in enumerate(chunks):
                            nc.tensor.matmul(
                                out=ps[:],
                                lhsT=indb[:, c, :],
                                rhs=mbt[:, c, :],
                                start=(j == 0),
                                stop=(j == len(chunks) - 1),
                            )
                        # ---- fused epilogue for this tile
                        S = ps[:, 0:256]
                        asum = ps[:, 256:260]
                        r4 = epip.tile([P, 4], F32, tag="r4")
                        nc.vector.tensor_scalar(
                            r4[:], asum, 4.0, TINY, OP.mult, OP.max
                        )
                        rec = epip.tile([P, 4], F32, tag="rec")
                        nc.vector.reciprocal(rec[:], r4[:])
                        m = epip.tile([P, HID], F32, tag="m")
                        nc.vector.tensor_scalar(
                            m[:], S[:, 0:64], rec[:, 0:1], None, OP.mult
                        )
                        for h in range(1, H):
                            nc.vector.scalar_tensor_tensor(
                                out=m[:],
                                in0=S[:, 64 * h : 64 * (h + 1)],
                                scalar=rec[:, h : h + 1],
                                in1=m[:],
                                op0=OP.mult,
                                op1=OP.add,
                            )
                        xr = epip.tile([P, HID], F32, tag="xr")
                        nc.vector.tensor_tensor(
                            out=xr[:], in0=m[:], in1=res_sb[:, t, :], op=OP.add
                        )
                        nc.vector.tensor_tensor(
                            out=xr[:], in0=xr[:], in1=resb, op=OP.add
                        )
                        # LayerNorm + ReLU
                        stat = epip.tile([P, 8], F32, tag="stat")
                        nc.vector.tensor_reduce(
                            out=stat[:, 0:1], in_=xr[:], axis=AX.X, op=OP.add
                        )
                        nc.vector.tensor_scalar_mul(
                            stat[:, 1:2], stat[:, 0:1], -1.0 / HID
                        )
                        xc = epip.tile([P, HID], F32, tag="xc")
                        nc.scalar.activation(
                            out=xc[:], in_=xr[:], func=AF.Identity,
                            bias=stat[:, 1:2],
                        )
                        sq = epip.tile([P, HID], F32, tag="sq")
                        nc.scalar.activation(
                            out=sq[:], in_=xc[:], func=AF.Square,
                            accum_out=stat[:, 2:3],
                        )
                        nc.scalar.activation(
                            out=stat[:, 3:4], in_=stat[:, 2:3], func=AF.Sqrt,
                            bias=eps_col, scale=1.0 / HID,
                        )
                        nc.vector.reciprocal(stat[:, 4:5], stat[:, 3:4])
                        y = epip.tile([P, HID], F32, tag="y")
                        nc.vector.scalar_tensor_tensor(
                            out=y[:], in0=xc[:], scalar=stat[:, 4:5], in1=g_ln,
                            op0=OP.mult, op1=OP.mult,
                        )
                        hpre = epip.tile([P, HID], F32, tag="hpre")
                        nc.vector.tensor_tensor(
                            out=hpre[:], in0=y[:], in1=b_ln, op=OP.add
                        )
                        ht = epip.tile([P, HID], BF16, tag="ht")
                        nc.scalar.activation(
                            out=ht[:], in_=hpre[:], func=AF.Relu
                        )
                        hT_ps = tps.tile([HID, P], BF16, tag="hT_ps")
                        nc.tensor.transpose(
                            out=hT_ps[:], in_=ht[:], identity=ident[:]
                        )
                        hT = epip.tile([HID, P], BF16, tag="hT")
                        nc.scalar.copy(out=hT[:], in_=hT_ps[:])
                        if li == 0:
                            nc.sync.dma_start(
                                out=h1ownT[:, t * P : (t + 1) * P], in_=hT[:]
                            )
                        else:
                            hd_ps = dps.tile([P, OUT], F32, tag="dc_ps")
                            nc.tensor.matmul(
                                out=hd_ps[:], lhsT=hT[:], rhs=predw[:],
                                start=True, stop=True,
                            )
                            ob = epip.tile([P, OUT], F32, tag="ob")
                            nc.vector.tensor_tensor(
                                out=ob[:], in0=hd_ps[:], in1=predb, op=OP.add
                            )
                            nc.sync.dma_start(
                                out=out_t[t * P : (t + 1) * P, :], in_=ob[:]
                            )

            # =============== schedule ===============
            stages = os.environ.get(
                "GAT_STAGES", "d0,o0,e0,c,d1,o1,e1"
            ).split(",")
            if "d0" in stages:
                dense_phase(0)
            if "o0" in stages:
                own_phase(0)
            if "e0" in stages:
                edge_phase(0)
            if "c" in stages:
                nc.gpsimd.collective_compute(
                    "AllGather",
                    OP.bypass,
                    replica_groups=[list(range(NCORES))],
                    ins=[h1ownT[:].opt()],
                    outs=[h1Tb[:].opt()],
                )
            if "d1" in stages:
                dense_phase(1)
            if "o1" in stages:
                own_phase(1)
            if "e1" in stages:
                edge_phase(1)
            if "e1" not in stages:
                # keep the output tensor written so PJRT returns
                dummy = epip.tile([P, OUT], F32, tag="ob")
                nc.vector.memset(dummy[:], 0.0)
                nc.sync.dma_start(out=out_t[0:P, :], in_=dummy[:])

    nc.compile()
    return nc


# ---------------------------------------------------------------------------
# Host entry point
# ---------------------------------------------------------------------------
def kernel(feats, src, dst, W0, al0, ar0, resw0, resb0, g0, b0,
           W1, al1, ar1, resw1, resb1, g1, b1, predw, predb):
    f32 = np.float32
    bf16 = ml_dtypes.bfloat16
    feats = np.asarray(feats, f32)
    W0 = np.asarray(W0, f32)
    al0 = np.asarray(al0, f32)
    ar0 = np.asarray(ar0, f32)
    resw0 = np.asarray(resw0, f32)
    W1 = np.asarray(W1, f32)
    al1 = np.asarray(al1, f32)
    ar1 = np.asarray(ar1, f32)
    resw1 = np.asarray(resw1, f32)
    predw_np = np.asarray(predw, f32)

    ep = _prep_edges(src, dst)
    nc = _build_program(ep)

    feats_pad = np.zeros((N_PAD, IN), f32)
    feats_pad[:N] = feats
    featsT = np.ascontiguousarray(feats_pad.T).astype(bf16)

    def fold(W, a):
        return (W.reshape(W.shape[0], H, HID) * a[None]).sum(-1)

    wcat0 = np.concatenate([W0, fold(W0, al0)], axis=1)
    wcat1 = np.concatenate([W1, fold(W1, al1)], axis=1)
    wres0 = np.concatenate([fold(W0, ar0), resw0], axis=1)
    wres1 = np.concatenate([fold(W1, ar1), resw1], axis=1)

    aux = np.zeros((P, 8 * 64), f32)
    aux[:, 0:64] = np.asarray(g0, f32)[None]
    aux[:, 64:128] = np.asarray(b0, f32)[None]
    aux[:, 128:192] = np.asarray(g1, f32)[None]
    aux[:, 192:256] = np.asarray(b1, f32)[None]
    aux[:, 256:320] = np.asarray(resb0, f32)[None]
    aux[:, 320:384] = np.asarray(resb1, f32)[None]
    aux[:, 384:448] = np.asarray(predb, f32)[None]
    aux[:, 448] = EPS

    ident = np.eye(P, dtype=f32).astype(bf16)

    shared = {
        "featsT": featsT,
        "wcat0": np.ascontiguousarray(wcat0).astype(bf16),
        "wcat1": np.ascontiguousarray(wcat1).astype(bf16),
        "wres0": np.ascontiguousarray(wres0).astype(bf16),
        "wres1": np.ascontiguousarray(wres1).astype(bf16),
        "predw": np.ascontiguousarray(predw_np).astype(bf16),
        "aux": aux,
        "ident": ident,
    }
    in_maps = [
        {
            **shared,
            "ownxT": np.ascontiguousarray(featsT[:, c * NPC : (c + 1) * NPC]),
            "idx16": ep["idx16"][c],
            "ind": ep["ind"][c],
            "indT": ep["indT"][c],
        }
        for c in range(NCORES)
    ]

    trace = os.environ.get("GAT_TRACE", "0") == "1"
    res = run_bass_kernel_spmd(
        nc, in_maps, core_ids=list(range(NCORES)), trace=trace
    )
    if trace and res.exec_time_ns is not None:
        print(f"HW exec time: {res.exec_time_ns} ns")
        if res.instructions_and_trace is not None:
            print(f"trace: {res.instructions_and_trace[1]}")

    out = np.concatenate([res.results[c]["out"] for c in range(NCORES)], axis=0)
    return np.ascontiguousarray(out[:N]).astype(np.float32)
